# revision 24
# baseline (speedup 1.0000x reference)
"""Trainium2 Bass kernel: EdgeModelConcat (GNN edge MLP).

reference math (per edge e):
    x   = concat([dest[e], src[e], u[batch[e]]])      # [192]
    h   = relu(x @ W1 + b1)                            # [256]
    out = h @ W2 + b2                                  # [64]
(edge_attr is an input but unused by the reference.)

Strategy
--------
Data-parallel over edges on 8 NeuronCores, all in bf16 with fp32 PSUM.
Host does all layout marshalling so the device only does DMAs + matmuls
+ fused bias/relu:

* host passes x^T = [dest^T; src^T] as a [128, E/8] bf16 array per core,
  so layer-1 is h^T = W1[:128].T @ x^T with K=128, no device transposes.
* the u-term is folded into a per-graph bias table computed on host:
  cT[g] = u[g] @ W1[128:] + b1 ([256, 512] f32, uploaded).  `batch` is
  sorted, so per 512-edge tile the bias column is piecewise constant;
  segment boundaries are baked into the instruction stream as static
  column ranges of the fused relu+bias ops.  Per-core segment structure
  differs -> one 8-way tc.Switch on partition_id.
* layer-2 accumulates the K=256 contraction as 2 matmuls per tile; even
  tiles write PSUM partitions 0:64 (PE col-group A), odd tiles 64:128
  (group B, via auto tile_position), so a pair of tiles lands as one
  [128, 512] PSUM tile evacuated in a single op and DMA'd at full
  partition width.  Layer-2 weight loads are shared across 2 pairs and
  layer-1 loads across 3 tiles (consecutive same-weight matmuls skip
  the ~107ns serialized LDWEIGHTS; measured, not just the docs' claim).
* dummy matmuls with no DMA dependency run at kernel start so the PE
  HAM clock-gate opens (1.2 -> 2.4 GHz) during the first slab load,
  which cannot land before ~13us (the DMA subsystem spends ~9.5us
  warming up and round-robins all queued transfers; ramp-phase slab
  loads are serialized via GpSimd gate ops so the first chunk is not
  starved by later slabs).
* outputs are stored bf16 and widened on the host.
"""

import numpy as np

MODE = "bf16"              # kept for test.py compat; only bf16 supported
PROFILE = False            # set True (with NTFF hook installed) to measure
LAST_EXEC_NS = None        # exec time of slowest profiled core, ns
LAST_RESULTS = None

NCORES = 8
TILE = 512                 # edges per matmul tile (PSUM bank = 512 f32)
SLAB_TILES = 8             # tiles per DMA slab (4096 edges = 1MB bf16 in)
WARMUP_MMS = 12            # dummy matmuls to open the HAM clock gate

_cache = {}


def _np_bf16():
    import ml_dtypes

    return np.dtype(ml_dtypes.bfloat16)


def _segments_per_tile(bk, ec, ntiles):
    """bk: per-core sorted graph ids [ec] -> list per tile of (a, b, g)."""
    out = []
    for t in range(ntiles):
        c0 = t * TILE
        w = min(TILE, ec - c0)
        vals = bk[c0 : c0 + w]
        bounds = np.flatnonzero(np.diff(vals)) + 1
        starts = np.concatenate([[0], bounds, [w]])
        out.append(
            [
                (int(starts[i]), int(starts[i + 1]), int(vals[starts[i]]))
                for i in range(len(starts) - 1)
            ]
        )
    return out


def _out_col(t):
    return (t // SLAB_TILES) * (SLAB_TILES // 2) * TILE + ((t % SLAB_TILES) // 2) * TILE


def _build(all_segs, ec, fx, fu, h, fo, b, out_w):
    from contextlib import ExitStack

    import concourse.bass as bass
    import concourse.mybir as mybir
    import concourse.tile as tile
    from concourse import bacc

    F32 = mybir.dt.float32
    BF16 = mybir.dt.bfloat16
    Relu = mybir.ActivationFunctionType.Relu
    Ident = mybir.ActivationFunctionType.Identity
    ADD = mybir.AluOpType.add
    MAX = mybir.AluOpType.max

    ntiles = (ec + TILE - 1) // TILE
    nslabs = (ntiles + SLAB_TILES - 1) // SLAB_TILES
    slab = TILE * SLAB_TILES
    kin = 2 * fx            # 128: contraction dim of layer 1
    mh = h // 128           # 2: H chunks of 128
    assert kin == 128 and h == 256 and fo <= 64

    nc = bacc.Bacc("TRN2", target_bir_lowering=False, debug=False, num_devices=NCORES)
    # cb (bf16) = [w1ds (h cols) | w2c (mh*fo cols)]
    # cbias (f32) = [cT chunk0 (b cols) | cT chunk1 (b cols) | b2 col]
    cb_w = h + mh * fo
    xT = nc.declare_dram_parameter("xT", [kin, ec], BF16, isOutput=False)
    cb = nc.declare_dram_parameter("cb", [128, cb_w], BF16, isOutput=False)
    cbias = nc.declare_dram_parameter("cbias", [128, mh * b + 1], F32, isOutput=False)
    outT = nc.declare_dram_parameter("outT", [128, out_w], BF16, isOutput=True)

    with tile.TileContext(nc) as tc, ExitStack() as ctx:
        pid = nc.partition_id()

        const = ctx.enter_context(tc.tile_pool(name="const", bufs=1))
        xp = ctx.enter_context(tc.tile_pool(name="xp", bufs=4))
        hp = ctx.enter_context(tc.tile_pool(name="hp", bufs=12))
        op = ctx.enter_context(tc.tile_pool(name="op", bufs=4))
        ph0 = ctx.enter_context(tc.tile_pool(name="ph0", bufs=3, space="PSUM"))
        ph1 = ctx.enter_context(tc.tile_pool(name="ph1", bufs=3, space="PSUM"))
        po = ctx.enter_context(tc.tile_pool(name="po", bufs=2, space="PSUM"))

        # PE warm-up: dummy matmuls with zero DMA dependencies keep the PE
        # busy from program start so the HAM clock-gate opens (~3.4us of
        # sustained activity) while the first input slab is still loading.
        wt = const.tile([128, TILE], BF16)
        nc.vector.memset(wt[:], 0.25)
        # dummy activation forces the ACT spline-table load (~1.3us) to
        # happen during the ramp instead of before the first real evac
        wact = const.tile([128, 64], BF16)
        nc.scalar.activation(wact[:], wt[:, 0:64], Relu, bias=0.0)
        for _ in range(WARMUP_MMS):
            wps = po.tile([128, TILE], F32, tag="o", name="wps")
            nc.tensor.matmul(wps[:], wt[:, 0:128], wt[:], start=True, stop=True)

        pre_slabs = {}   # slab index -> (in tile, out tile), common code

        def emit_slab_load(s, chunks=1, gate=None):
            # chunks>1 splits the load so the first tiles' data lands early.
            # gate=prev_xt serializes ramp-phase loads: DMA rings round-robin
            # across ALL queued transfers, so without a gate the first tile's
            # data crawls at 1/N of line rate behind later slabs.  The gate
            # is a 1-elem GpSimd copy (waits for the previous slab) followed
            # by a 1-elem memset into the new tile (WAW makes this slab's
            # DMA wait) -- GpSimd is otherwise idle, so nothing else blocks.
            c0 = s * slab
            ws = min(slab, ec - c0)
            xtn = xp.tile([kin, slab], BF16, tag="xt", name="xt")
            if gate is not None:
                gt = const.tile([1, 1], BF16)
                nc.gpsimd.tensor_copy(gt[:], gate[0:1, slab - 1 : slab])
                nc.gpsimd.memset(xtn[0:1, 0:1], 0)
            step = (slab // chunks + TILE - 1) // TILE * TILE
            for q0 in range(0, ws, step):
                q1 = min(q0 + step, ws)
                nc.sync.dma_start(
                    xtn[:, q0:q1], xT[:, c0 + q0 : c0 + q1]
                )
            otn = op.tile([128, slab // 2], BF16, tag="ot", name="ot")
            return (xtn, otn)

        # DMA issue order sets arrival order: weights (needed by the first
        # matmul), first input slab, bias table (needed ~1us later by the
        # first evac), then the prefetch slab.  Slabs 0/1 load in common
        # code; later slabs are issued inside each core's Switch branch.
        cb_sb = const.tile([128, cb_w], BF16)
        nc.sync.dma_start(cb_sb[:], cb[:])
        pre_slabs[0] = emit_slab_load(0, chunks=4)
        cbias_sb = const.tile([128, mh * b + 1], F32)
        nc.sync.dma_start(cbias_sb[:], cbias[:])
        if nslabs > 1:
            pre_slabs[1] = emit_slab_load(1, chunks=2, gate=pre_slabs[0][0])
        w1ds_sb = cb_sb[:, 0:h]
        w2c_sb = cb_sb[:, h : h + mh * fo]
        b2c_sb = cbias_sb[:, mh * b : mh * b + 1]

        for core in tc.Switch(pid, NCORES):
            segs_per_tile = all_segs[core]
            hss = {}       # tile index -> relu'd h tile (sbuf)
            widths = {}
            xts = {s: v[0] for s, v in pre_slabs.items()}
            ots = {s: v[1] for s, v in pre_slabs.items()}

            def load_slab(s):
                if s in xts or s >= nslabs:
                    return
                g = xts[s - 1] if s in (2, 3) else None
                xts[s], ots[s] = emit_slab_load(s, chunks=2, gate=g)

            def store_slab(tp, w, ot):
                # store in half-slabs (4 tiles = 256KB) so the final store
                # has less data queued behind it; the last slab stores per
                # pair so the tail drains as early as possible.  Stores go
                # on the ACT HWDGE ring so they never head-of-line-block
                # input slab loads on the SP ring (HWDGE completes FIFO
                # per ring).
                oc0 = (tp // SLAB_TILES) * (slab // 2)
                j = tp % SLAB_TILES
                if tp // SLAB_TILES == nslabs - 1:
                    cc = (j // 2) * TILE
                    if tp % 2 == 1:
                        nc.scalar.dma_start(
                            outT[:, oc0 + cc : oc0 + cc + TILE],
                            ot[:, cc : cc + TILE],
                        )
                    elif tp == ntiles - 1:
                        nc.scalar.dma_start(
                            outT[0:fo, oc0 + cc : oc0 + cc + w],
                            ot[0:fo, cc : cc + w],
                        )
                    return
                if j == 3:
                    nc.scalar.dma_start(
                        outT[:, oc0 : oc0 + 2 * TILE], ot[:, 0 : 2 * TILE]
                    )
                elif j == SLAB_TILES - 1:
                    nc.scalar.dma_start(
                        outT[:, oc0 + 2 * TILE : oc0 + 4 * TILE],
                        ot[:, 2 * TILE : 4 * TILE],
                    )
                elif tp == ntiles - 1:
                    lo = 2 * TILE if j >= 4 else 0
                    nt = j + 1
                    hi = (nt // 2) * TILE
                    if hi > lo:
                        nc.scalar.dma_start(
                            outT[:, oc0 + lo : oc0 + hi], ot[:, lo:hi]
                        )
                    if nt % 2:
                        nc.scalar.dma_start(
                            outT[0:fo, oc0 + hi : oc0 + hi + w],
                            ot[0:fo, hi : hi + w],
                        )

            # L1 for a group of 3 tiles: same-weight matmuls adjacent so the
            # walrus ldw-opt elides redundant LDWEIGHTS; relu+bias evac
            # split DVE (h0) / ACT (h1) per static batch segments.
            def emit_l1(group):
                tiles = []
                for t in group:
                    s, j = divmod(t, SLAB_TILES)
                    load_slab(s)
                    if j == 0:
                        load_slab(s + 1)   # prefetch a full slab ahead
                    if j == 4:
                        load_slab(s + 2)   # deepen prefetch mid-slab
                    xtt = xts[s]
                    a = j * TILE
                    w = min(TILE, ec - t * TILE)
                    widths[t] = w
                    h0 = ph0.tile([128, TILE], F32, tag="h0", name="h0")
                    h1 = ph1.tile([128, TILE], F32, tag="h1", name="h1")
                    tiles.append((t, xtt, a, w, h0, h1))
                for (t, xtt, a, w, h0, h1) in tiles:
                    nc.tensor.matmul(
                        h0[:, :w], w1ds_sb[:, 0:128], xtt[:, a : a + w],
                        start=True, stop=True,
                    )
                for (t, xtt, a, w, h0, h1) in tiles:
                    nc.tensor.matmul(
                        h1[:, :w], w1ds_sb[:, 128:256], xtt[:, a : a + w],
                        start=True, stop=True,
                    )
                for (t, xtt, a, w, h0, h1) in tiles:
                    hs = hp.tile([128, 2 * TILE], BF16, tag="hs", name="hs")
                    hss[t] = hs
                    for (sa, sb, g) in segs_per_tile[t]:
                        nc.vector.tensor_scalar(
                            out=hs[:, sa:sb], in0=h0[:, sa:sb],
                            scalar1=cbias_sb[:, g : g + 1], scalar2=0.0,
                            op0=ADD, op1=MAX,
                        )
                        nc.scalar.activation(
                            hs[:, TILE + sa : TILE + sb], h1[:, sa:sb], Relu,
                            bias=cbias_sb[:, b + g : b + g + 1],
                        )

            # L2 for a group of <=4 tiles (2 pairs): even tiles -> PSUM
            # partitions 0:64 (PE col-group A), odd -> 64:128 (group B).
            # Same-weight matmuls adjacent across the 2 pairs halve the
            # LDWEIGHTS count; accumulation pairs stay contiguous per
            # col-group (interleaving start/stop across groups serializes
            # the scheduler badly).
            def emit_l2(group):
                o_of = {}
                for p in range(0, len(group), 2):
                    opair = po.tile([128, TILE], F32, tag="o", name="o_pair")
                    for t in group[p : p + 2]:
                        o_of[t] = opair
                for r0, par in ((0, 0), (fo, 1)):       # col-group A, then B
                    gts = [t for t in group if t % 2 == par]
                    for m in range(mh):                 # W2 K-chunk 0, then 1
                        for t in gts:
                            w = widths[t]
                            nc.tensor.matmul(
                                o_of[t][r0 : r0 + fo, :w],
                                w2c_sb[:, m * fo : (m + 1) * fo],
                                hss[t][:, m * TILE : m * TILE + w],
                                start=(m == 0), stop=(m == mh - 1),
                            )
                for p in range(0, len(group), 2):
                    pt = group[p : p + 2]
                    t0 = pt[0]
                    full = len(pt) == 2
                    w = widths[pt[-1]]
                    for t in pt:
                        hss.pop(t), widths.pop(t)
                    s = t0 // SLAB_TILES
                    cc = ((t0 % SLAB_TILES) // 2) * TILE
                    ot = ots[s]
                    opair = o_of[t0]
                    if full:
                        src = opair[:]
                        dst = ot[:, cc : cc + TILE]
                    else:
                        src = opair[0:fo, :w]
                        dst = ot[0:fo, cc : cc + w]
                    if (t0 // 2) % 2 == 0:
                        nc.vector.tensor_scalar(
                            out=dst, in0=src,
                            scalar1=b2c_sb[:] if full else b2c_sb[0:fo, :],
                            scalar2=None, op0=ADD,
                        )
                    else:
                        nc.scalar.activation(
                            dst, src, Ident,
                            bias=b2c_sb[:] if full else b2c_sb[0:fo, :],
                        )
                    store_slab(pt[-1], w, ot)

            # L1 in groups of 3 (fits ph0/ph1 bufs, 2 LDW per 3 tiles);
            # L2 in groups of 4, lagging L1 by >=3 tiles for pipelining.
            l1_groups = [
                list(range(q, min(q + 3, ntiles))) for q in range(0, ntiles, 3)
            ]
            l2_groups = [
                list(range(q, min(q + 4, ntiles))) for q in range(0, ntiles, 4)
            ]
            next_l2 = 0
            for grp in l1_groups:
                emit_l1(grp)
                done = grp[-1] + 1
                while (
                    next_l2 < len(l2_groups)
                    and l2_groups[next_l2][-1] + 2 <= done
                ):
                    emit_l2(l2_groups[next_l2])
                    next_l2 += 1
            while next_l2 < len(l2_groups):
                emit_l2(l2_groups[next_l2])
                next_l2 += 1
    nc.compile()
    return nc


def kernel(**inputs):
    global LAST_EXEC_NS, LAST_RESULTS

    src = np.asarray(inputs["src"], dtype=np.float32)
    dest = np.asarray(inputs["dest"], dtype=np.float32)
    u = np.asarray(inputs["u"], dtype=np.float32)
    batch = np.asarray(inputs["batch"])
    W1 = np.asarray(inputs["W1"], dtype=np.float32)
    b1 = np.asarray(inputs["b1"], dtype=np.float32)
    W2 = np.asarray(inputs["W2"], dtype=np.float32)
    b2 = np.asarray(inputs["b2"], dtype=np.float32)

    e, fx = src.shape
    b_, fu = u.shape
    h = W1.shape[1]
    fo = W2.shape[1]
    ec = (e + NCORES - 1) // NCORES
    ntiles = (ec + TILE - 1) // TILE
    mh = h // 128
    bf16 = _np_bf16()

    # sorted edge order (identity when batch already sorted, as speced)
    bi = batch.astype(np.int64)
    if np.any(bi[1:] < bi[:-1]):
        perm = np.argsort(bi, kind="stable")
    else:
        perm = None
    bs = bi if perm is None else bi[perm]

    # host-side marshalling ------------------------------------------------
    # out column layout: tile t -> cols [S(t), S(t)+w) rows (t%2)*64
    wlast = ec - (ntiles - 1) * TILE
    out_w = max(_out_col(ntiles - 1) + wlast, _out_col(max(ntiles - 2, 0)) + TILE)

    # weights: cb = [W1[:128] | W2 chunks], bf16
    cb = np.concatenate(
        [W1[: 2 * fx]]
        + [np.concatenate([W2[i * 128 : (i + 1) * 128] for i in range(mh)], axis=1)],
        axis=1,
    ).astype(bf16)
    cb = np.ascontiguousarray(cb)

    # bias table: cT[g] = u[g] @ W1[128:] + b1, plus a b2 column, f32
    cT = u @ W1[2 * fx :] + b1                    # [b, h]
    cbias = np.zeros((128, mh * b_ + 1), dtype=np.float32)
    for m in range(mh):
        cbias[:, m * b_ : (m + 1) * b_] = cT[:, m * 128 : (m + 1) * 128].T
    cbias[:, mh * b_] = np.tile(b2, mh)[:128]

    all_segs = []
    in_maps = []
    for k in range(NCORES):
        i0, i1 = k * ec, min((k + 1) * ec, e)
        n = i1 - i0
        if perm is None:
            d_k = dest[i0:i1]
            s_k = src[i0:i1]
        else:
            idx = perm[i0:i1]
            d_k = dest[idx]
            s_k = src[idx]
        xTk = np.empty((2 * fx, ec), dtype=bf16)
        xTk[:fx, :n] = d_k.T
        xTk[fx:, :n] = s_k.T
        if n < ec:
            xTk[:, n:] = 0
        bk = np.empty(ec, dtype=np.int64)
        bk[:n] = bs[i0:i1]
        if n < ec:
            bk[n:] = bk[n - 1]
        all_segs.append(_segments_per_tile(bk, ec, ntiles))
        in_maps.append({"xT": xTk, "cb": cb, "cbias": cbias})

    # build / fetch compiled program --------------------------------------
    key = (e, fx, fu, h, fo, b_, hash(bs.tobytes()))
    nc = _cache.get(key)
    if nc is None:
        nc = _build(all_segs, ec, fx, fu, h, fo, b_, out_w)
        _cache.clear()
        _cache[key] = nc

    from concourse.bass_utils import run_bass_kernel_spmd

    res = run_bass_kernel_spmd(
        nc, in_maps, list(range(NCORES)), trace=bool(PROFILE)
    )
    LAST_EXEC_NS = res.exec_time_ns
    LAST_RESULTS = res

    # unpack ---------------------------------------------------------------
    out = np.empty((e, fo), dtype=np.float32)
    for k in range(NCORES):
        o = np.asarray(res.results[k]["outT"]).astype(np.float32)
        i0, i1 = k * ec, min((k + 1) * ec, e)
        n = i1 - i0
        ok = np.empty((ec, fo), dtype=np.float32)
        for t in range(ntiles):
            w = min(TILE, ec - t * TILE)
            c = _out_col(t)
            r = (t % 2) * 64
            ok[t * TILE : t * TILE + w] = o[r : r + fo, c : c + w].T
        if perm is None:
            out[i0:i1] = ok[:n]
        else:
            out[perm[i0:i1]] = ok[:n]
    return out


if __name__ == "__main__":
    # small self-test with synthetic inputs (E scaled down)
    rng = np.random.default_rng(0)
    E, FX, FU, H, FO, B = 40960, 64, 64, 256, 64, 512
    src = rng.standard_normal((E, FX), dtype=np.float32)
    dest = rng.standard_normal((E, FX), dtype=np.float32)
    u = rng.standard_normal((B, FU), dtype=np.float32)
    batch = np.sort(rng.integers(0, B, E)).astype(np.int64)
    W1 = (rng.standard_normal((2 * FX + FU, H), dtype=np.float32) / np.sqrt(2 * FX + FU))
    b1 = np.zeros(H, np.float32)
    W2 = rng.standard_normal((H, FO), dtype=np.float32) / np.sqrt(H)
    b2 = rng.standard_normal(FO, dtype=np.float32)
    got = kernel(src=src, dest=dest, edge_attr=src, u=u, batch=batch,
                 W1=W1, b1=b1, W2=W2, b2=b2)
    x = np.concatenate([dest, src, u[batch]], axis=1)
    hh = np.maximum(x @ W1 + b1, 0.0)
    want = hh @ W2 + b2
    rel = np.linalg.norm(got - want) / np.linalg.norm(want)
    print("rel err:", rel)


# revision 25
# speedup vs baseline: 1.1799x; 1.1799x over previous
"""Trainium2 Bass kernel: EdgeModelConcat (GNN edge MLP).

reference math (per edge e):
    x   = concat([dest[e], src[e], u[batch[e]]])      # [192]
    h   = relu(x @ W1 + b1)                            # [256]
    out = h @ W2 + b2                                  # [64]
(edge_attr is an input but unused by the reference.)

Strategy
--------
Data-parallel over edges on 8 NeuronCores, all in bf16 with fp32 PSUM.
Host does all layout marshalling so the device only does DMAs + matmuls
+ fused bias/relu:

* host passes x^T = [dest^T; src^T] as a [128, E/8] bf16 array per core,
  so layer-1 is h^T = W1[:128].T @ x^T with K=128, no device transposes.
* the u-term is folded into a per-graph bias table computed on host:
  cT[g] = u[g] @ W1[128:] + b1 ([256, 512] f32, uploaded).  `batch` is
  sorted, so per 512-edge tile the bias column is piecewise constant;
  segment boundaries are baked into the instruction stream as static
  column ranges of the fused relu+bias ops.  Per-core segment structure
  differs -> one 8-way tc.Switch on partition_id.
* layer-2 accumulates the K=256 contraction as 2 matmuls per tile; even
  tiles write PSUM partitions 0:64 (PE col-group A), odd tiles 64:128
  (group B, via auto tile_position), so a pair of tiles lands as one
  [128, 512] PSUM tile evacuated in a single op and DMA'd at full
  partition width.  Layer-2 weight loads are shared across 2 pairs and
  layer-1 loads across 3 tiles (consecutive same-weight matmuls skip
  the ~107ns serialized LDWEIGHTS; measured, not just the docs' claim).
* dummy matmuls with no DMA dependency run at kernel start so the PE
  HAM clock-gate opens (1.2 -> 2.4 GHz) during the first slab load,
  which cannot land before ~13us (the DMA subsystem spends ~9.5us
  warming up and round-robins all queued transfers; ramp-phase slab
  loads are serialized via GpSimd gate ops so the first chunk is not
  starved by later slabs).
* outputs are stored bf16 and widened on the host.
"""

import numpy as np

MODE = "bf16"              # kept for test.py compat; only bf16 supported
PROFILE = False            # set True (with NTFF hook installed) to measure
LAST_EXEC_NS = None        # exec time of slowest profiled core, ns
LAST_RESULTS = None

NCORES = 8
TILE = 512                 # edges per matmul tile (PSUM bank = 512 f32)
SLAB_TILES = 8             # tiles per DMA slab (4096 edges = 1MB bf16 in)
WARMUP_MMS = 12            # dummy matmuls to open the HAM clock gate

_cache = {}


def _np_bf16():
    import ml_dtypes

    return np.dtype(ml_dtypes.bfloat16)


def _segments_per_tile(bk, ec, ntiles):
    """bk: per-core sorted graph ids [ec] -> list per tile of (a, b, g)."""
    out = []
    for t in range(ntiles):
        c0 = t * TILE
        w = min(TILE, ec - c0)
        vals = bk[c0 : c0 + w]
        bounds = np.flatnonzero(np.diff(vals)) + 1
        starts = np.concatenate([[0], bounds, [w]])
        out.append(
            [
                (int(starts[i]), int(starts[i + 1]), int(vals[starts[i]]))
                for i in range(len(starts) - 1)
            ]
        )
    return out


def _out_col(t):
    return (t // SLAB_TILES) * (SLAB_TILES // 2) * TILE + ((t % SLAB_TILES) // 2) * TILE


def _build(all_segs, ec, fx, fu, h, fo, b, out_w):
    from contextlib import ExitStack

    import concourse.bass as bass
    import concourse.mybir as mybir
    import concourse.tile as tile
    from concourse import bacc

    F32 = mybir.dt.float32
    BF16 = mybir.dt.bfloat16
    Relu = mybir.ActivationFunctionType.Relu
    Ident = mybir.ActivationFunctionType.Identity
    ADD = mybir.AluOpType.add
    MAX = mybir.AluOpType.max

    ntiles = (ec + TILE - 1) // TILE
    nslabs = (ntiles + SLAB_TILES - 1) // SLAB_TILES
    slab = TILE * SLAB_TILES
    kin = 2 * fx            # 128: contraction dim of layer 1
    mh = h // 128           # 2: H chunks of 128
    assert kin == 128 and h == 256 and fo <= 64

    nc = bacc.Bacc("TRN2", target_bir_lowering=False, debug=False, num_devices=NCORES)
    # cb (bf16) = [w1ds (h cols) | w2c (mh*fo cols)]
    # cbias (f32) = [cT chunk0 (b cols) | cT chunk1 (b cols) | b2 col]
    cb_w = h + mh * fo
    xT = nc.declare_dram_parameter("xT", [kin, ec], BF16, isOutput=False)
    cb = nc.declare_dram_parameter("cb", [128, cb_w], BF16, isOutput=False)
    cbias = nc.declare_dram_parameter("cbias", [128, mh * b + 1], F32, isOutput=False)
    outT = nc.declare_dram_parameter("outT", [128, out_w], BF16, isOutput=True)

    with tile.TileContext(nc) as tc, ExitStack() as ctx:
        pid = nc.partition_id()

        const = ctx.enter_context(tc.tile_pool(name="const", bufs=1))
        xp = ctx.enter_context(tc.tile_pool(name="xp", bufs=4))
        hp = ctx.enter_context(tc.tile_pool(name="hp", bufs=12))
        op = ctx.enter_context(tc.tile_pool(name="op", bufs=4))
        ph0 = ctx.enter_context(tc.tile_pool(name="ph0", bufs=3, space="PSUM"))
        ph1 = ctx.enter_context(tc.tile_pool(name="ph1", bufs=3, space="PSUM"))
        po = ctx.enter_context(tc.tile_pool(name="po", bufs=2, space="PSUM"))

        # PE warm-up: dummy matmuls with zero DMA dependencies keep the PE
        # busy from program start so the HAM clock-gate opens (~3.4us of
        # sustained activity) while the first input slab is still loading.
        wt = const.tile([128, TILE], BF16)
        nc.vector.memset(wt[:], 0.25)
        # dummy activation forces the ACT spline-table load (~1.3us) to
        # happen during the ramp instead of before the first real evac
        wact = const.tile([128, 64], BF16)
        nc.scalar.activation(wact[:], wt[:, 0:64], Relu, bias=0.0)
        for _ in range(WARMUP_MMS):
            wps = po.tile([128, TILE], F32, tag="o", name="wps")
            nc.tensor.matmul(wps[:], wt[:, 0:128], wt[:], start=True, stop=True)

        pre_slabs = {}   # slab index -> (in tile, out tile), common code

        def emit_slab_load(s, chunks=1, gate=None):
            # chunks>1 splits the load so the first tiles' data lands early.
            # gate=prev_xt serializes ramp-phase loads: DMA rings round-robin
            # across ALL queued transfers, so without a gate the first tile's
            # data crawls at 1/N of line rate behind later slabs.  The gate
            # is a 1-elem GpSimd copy (waits for the previous slab) followed
            # by a 1-elem memset into the new tile (WAW makes this slab's
            # DMA wait) -- GpSimd is otherwise idle, so nothing else blocks.
            c0 = s * slab
            ws = min(slab, ec - c0)
            xtn = xp.tile([kin, slab], BF16, tag="xt", name="xt")
            if gate is not None:
                gt = const.tile([1, 1], BF16)
                nc.gpsimd.tensor_copy(gt[:], gate[0:1, slab - 1 : slab])
                nc.gpsimd.memset(xtn[0:1, 0:1], 0)
            step = (slab // chunks + TILE - 1) // TILE * TILE
            for q0 in range(0, ws, step):
                q1 = min(q0 + step, ws)
                nc.sync.dma_start(
                    xtn[:, q0:q1], xT[:, c0 + q0 : c0 + q1]
                )
            otn = op.tile([128, slab // 2], BF16, tag="ot", name="ot")
            return (xtn, otn)

        # DMA issue order sets arrival order: weights (needed by the first
        # matmul), first input slab, bias table (needed ~1us later by the
        # first evac), then the prefetch slab.  Slabs 0/1 load in common
        # code; later slabs are issued inside each core's Switch branch.
        cb_sb = const.tile([128, cb_w], BF16)
        nc.sync.dma_start(cb_sb[:], cb[:])
        pre_slabs[0] = emit_slab_load(0, chunks=4)
        cbias_sb = const.tile([128, mh * b + 1], F32)
        nc.sync.dma_start(cbias_sb[:], cbias[:])
        if nslabs > 1:
            pre_slabs[1] = emit_slab_load(1, chunks=2, gate=pre_slabs[0][0])
        w1ds_sb = cb_sb[:, 0:h]
        w2c_sb = cb_sb[:, h : h + mh * fo]
        b2c_sb = cbias_sb[:, mh * b : mh * b + 1]

        for core in tc.Switch(pid, NCORES):
            segs_per_tile = all_segs[core]
            hss = {}       # tile index -> relu'd h tile (sbuf)
            widths = {}
            xts = {s: v[0] for s, v in pre_slabs.items()}
            ots = {s: v[1] for s, v in pre_slabs.items()}

            def load_slab(s):
                if s in xts or s >= nslabs:
                    return
                g = xts[s - 1] if s in (2, 3) else None
                xts[s], ots[s] = emit_slab_load(s, chunks=2, gate=g)

            def store_slab(tp, w, ot):
                # store in half-slabs (4 tiles = 256KB) so the final store
                # has less data queued behind it; the last slab stores per
                # pair so the tail drains as early as possible.  Stores go
                # via GpSimd SWDGE (own ring + idle engine FIFO) so they
                # neither head-of-line-block input slab loads on the SP
                # ring nor stall ACT behind cross-engine evac waits.
                oc0 = (tp // SLAB_TILES) * (slab // 2)
                j = tp % SLAB_TILES
                if tp // SLAB_TILES == nslabs - 1:
                    cc = (j // 2) * TILE
                    if tp % 2 == 1:
                        nc.gpsimd.dma_start(
                            outT[:, oc0 + cc : oc0 + cc + TILE],
                            ot[:, cc : cc + TILE],
                        )
                    elif tp == ntiles - 1:
                        nc.gpsimd.dma_start(
                            outT[0:fo, oc0 + cc : oc0 + cc + w],
                            ot[0:fo, cc : cc + w],
                        )
                    return
                if j == 3:
                    nc.gpsimd.dma_start(
                        outT[:, oc0 : oc0 + 2 * TILE], ot[:, 0 : 2 * TILE]
                    )
                elif j == SLAB_TILES - 1:
                    nc.gpsimd.dma_start(
                        outT[:, oc0 + 2 * TILE : oc0 + 4 * TILE],
                        ot[:, 2 * TILE : 4 * TILE],
                    )
                elif tp == ntiles - 1:
                    lo = 2 * TILE if j >= 4 else 0
                    nt = j + 1
                    hi = (nt // 2) * TILE
                    if hi > lo:
                        nc.gpsimd.dma_start(
                            outT[:, oc0 + lo : oc0 + hi], ot[:, lo:hi]
                        )
                    if nt % 2:
                        nc.gpsimd.dma_start(
                            outT[0:fo, oc0 + hi : oc0 + hi + w],
                            ot[0:fo, hi : hi + w],
                        )

            # L1 for a group of 3 tiles: same-weight matmuls adjacent so the
            # walrus ldw-opt elides redundant LDWEIGHTS; relu+bias evac
            # split DVE (h0) / ACT (h1) per static batch segments.
            def emit_l1(group):
                tiles = []
                for t in group:
                    s, j = divmod(t, SLAB_TILES)
                    load_slab(s)
                    if j == 0:
                        load_slab(s + 1)   # prefetch a full slab ahead
                    if j == 4:
                        load_slab(s + 2)   # deepen prefetch mid-slab
                    xtt = xts[s]
                    a = j * TILE
                    w = min(TILE, ec - t * TILE)
                    widths[t] = w
                    h0 = ph0.tile([128, TILE], F32, tag="h0", name="h0")
                    h1 = ph1.tile([128, TILE], F32, tag="h1", name="h1")
                    tiles.append((t, xtt, a, w, h0, h1))
                for (t, xtt, a, w, h0, h1) in tiles:
                    nc.tensor.matmul(
                        h0[:, :w], w1ds_sb[:, 0:128], xtt[:, a : a + w],
                        start=True, stop=True,
                    )
                for (t, xtt, a, w, h0, h1) in tiles:
                    nc.tensor.matmul(
                        h1[:, :w], w1ds_sb[:, 128:256], xtt[:, a : a + w],
                        start=True, stop=True,
                    )
                for (t, xtt, a, w, h0, h1) in tiles:
                    hs = hp.tile([128, 2 * TILE], BF16, tag="hs", name="hs")
                    hss[t] = hs
                    for (sa, sb, g) in segs_per_tile[t]:
                        nc.vector.tensor_scalar(
                            out=hs[:, sa:sb], in0=h0[:, sa:sb],
                            scalar1=cbias_sb[:, g : g + 1], scalar2=0.0,
                            op0=ADD, op1=MAX,
                        )
                        nc.scalar.activation(
                            hs[:, TILE + sa : TILE + sb], h1[:, sa:sb], Relu,
                            bias=cbias_sb[:, b + g : b + g + 1],
                        )

            # L2 for a group of <=4 tiles (2 pairs): even tiles -> PSUM
            # partitions 0:64 (PE col-group A), odd -> 64:128 (group B).
            # Same-weight matmuls adjacent across the 2 pairs halve the
            # LDWEIGHTS count; accumulation pairs stay contiguous per
            # col-group (interleaving start/stop across groups serializes
            # the scheduler badly).
            def emit_l2(group):
                o_of = {}
                for p in range(0, len(group), 2):
                    opair = po.tile([128, TILE], F32, tag="o", name="o_pair")
                    for t in group[p : p + 2]:
                        o_of[t] = opair
                for r0, par in ((0, 0), (fo, 1)):       # col-group A, then B
                    gts = [t for t in group if t % 2 == par]
                    for m in range(mh):                 # W2 K-chunk 0, then 1
                        for t in gts:
                            w = widths[t]
                            nc.tensor.matmul(
                                o_of[t][r0 : r0 + fo, :w],
                                w2c_sb[:, m * fo : (m + 1) * fo],
                                hss[t][:, m * TILE : m * TILE + w],
                                start=(m == 0), stop=(m == mh - 1),
                            )
                for p in range(0, len(group), 2):
                    pt = group[p : p + 2]
                    t0 = pt[0]
                    full = len(pt) == 2
                    w = widths[pt[-1]]
                    for t in pt:
                        hss.pop(t), widths.pop(t)
                    s = t0 // SLAB_TILES
                    cc = ((t0 % SLAB_TILES) // 2) * TILE
                    ot = ots[s]
                    opair = o_of[t0]
                    if full:
                        src = opair[:]
                        dst = ot[:, cc : cc + TILE]
                    else:
                        src = opair[0:fo, :w]
                        dst = ot[0:fo, cc : cc + w]
                    if (t0 // 2) % 2 == 0:
                        nc.vector.tensor_scalar(
                            out=dst, in0=src,
                            scalar1=b2c_sb[:] if full else b2c_sb[0:fo, :],
                            scalar2=None, op0=ADD,
                        )
                    else:
                        nc.scalar.activation(
                            dst, src, Ident,
                            bias=b2c_sb[:] if full else b2c_sb[0:fo, :],
                        )
                    store_slab(pt[-1], w, ot)

            # L1 in groups of 3 (fits ph0/ph1 bufs, 2 LDW per 3 tiles);
            # L2 in groups of 4, lagging L1 by >=3 tiles for pipelining.
            l1_groups = [
                list(range(q, min(q + 3, ntiles))) for q in range(0, ntiles, 3)
            ]
            l2_groups = [
                list(range(q, min(q + 4, ntiles))) for q in range(0, ntiles, 4)
            ]
            next_l2 = 0
            for grp in l1_groups:
                emit_l1(grp)
                done = grp[-1] + 1
                while (
                    next_l2 < len(l2_groups)
                    and l2_groups[next_l2][-1] + 2 <= done
                ):
                    emit_l2(l2_groups[next_l2])
                    next_l2 += 1
            while next_l2 < len(l2_groups):
                emit_l2(l2_groups[next_l2])
                next_l2 += 1
    nc.compile()
    return nc


def kernel(**inputs):
    global LAST_EXEC_NS, LAST_RESULTS

    src = np.asarray(inputs["src"], dtype=np.float32)
    dest = np.asarray(inputs["dest"], dtype=np.float32)
    u = np.asarray(inputs["u"], dtype=np.float32)
    batch = np.asarray(inputs["batch"])
    W1 = np.asarray(inputs["W1"], dtype=np.float32)
    b1 = np.asarray(inputs["b1"], dtype=np.float32)
    W2 = np.asarray(inputs["W2"], dtype=np.float32)
    b2 = np.asarray(inputs["b2"], dtype=np.float32)

    e, fx = src.shape
    b_, fu = u.shape
    h = W1.shape[1]
    fo = W2.shape[1]
    ec = (e + NCORES - 1) // NCORES
    ntiles = (ec + TILE - 1) // TILE
    mh = h // 128
    bf16 = _np_bf16()

    # sorted edge order (identity when batch already sorted, as speced)
    bi = batch.astype(np.int64)
    if np.any(bi[1:] < bi[:-1]):
        perm = np.argsort(bi, kind="stable")
    else:
        perm = None
    bs = bi if perm is None else bi[perm]

    # host-side marshalling ------------------------------------------------
    # out column layout: tile t -> cols [S(t), S(t)+w) rows (t%2)*64
    wlast = ec - (ntiles - 1) * TILE
    out_w = max(_out_col(ntiles - 1) + wlast, _out_col(max(ntiles - 2, 0)) + TILE)

    # weights: cb = [W1[:128] | W2 chunks], bf16
    cb = np.concatenate(
        [W1[: 2 * fx]]
        + [np.concatenate([W2[i * 128 : (i + 1) * 128] for i in range(mh)], axis=1)],
        axis=1,
    ).astype(bf16)
    cb = np.ascontiguousarray(cb)

    # bias table: cT[g] = u[g] @ W1[128:] + b1, plus a b2 column, f32
    cT = u @ W1[2 * fx :] + b1                    # [b, h]
    cbias = np.zeros((128, mh * b_ + 1), dtype=np.float32)
    for m in range(mh):
        cbias[:, m * b_ : (m + 1) * b_] = cT[:, m * 128 : (m + 1) * 128].T
    cbias[:, mh * b_] = np.tile(b2, mh)[:128]

    all_segs = []
    in_maps = []
    for k in range(NCORES):
        i0, i1 = k * ec, min((k + 1) * ec, e)
        n = i1 - i0
        if perm is None:
            d_k = dest[i0:i1]
            s_k = src[i0:i1]
        else:
            idx = perm[i0:i1]
            d_k = dest[idx]
            s_k = src[idx]
        xTk = np.empty((2 * fx, ec), dtype=bf16)
        xTk[:fx, :n] = d_k.T
        xTk[fx:, :n] = s_k.T
        if n < ec:
            xTk[:, n:] = 0
        bk = np.empty(ec, dtype=np.int64)
        bk[:n] = bs[i0:i1]
        if n < ec:
            bk[n:] = bk[n - 1]
        all_segs.append(_segments_per_tile(bk, ec, ntiles))
        in_maps.append({"xT": xTk, "cb": cb, "cbias": cbias})

    # build / fetch compiled program --------------------------------------
    key = (e, fx, fu, h, fo, b_, hash(bs.tobytes()))
    nc = _cache.get(key)
    if nc is None:
        nc = _build(all_segs, ec, fx, fu, h, fo, b_, out_w)
        _cache.clear()
        _cache[key] = nc

    from concourse.bass_utils import run_bass_kernel_spmd

    res = run_bass_kernel_spmd(
        nc, in_maps, list(range(NCORES)), trace=bool(PROFILE)
    )
    LAST_EXEC_NS = res.exec_time_ns
    LAST_RESULTS = res

    # unpack ---------------------------------------------------------------
    out = np.empty((e, fo), dtype=np.float32)
    for k in range(NCORES):
        o = np.asarray(res.results[k]["outT"]).astype(np.float32)
        i0, i1 = k * ec, min((k + 1) * ec, e)
        n = i1 - i0
        ok = np.empty((ec, fo), dtype=np.float32)
        for t in range(ntiles):
            w = min(TILE, ec - t * TILE)
            c = _out_col(t)
            r = (t % 2) * 64
            ok[t * TILE : t * TILE + w] = o[r : r + fo, c : c + w].T
        if perm is None:
            out[i0:i1] = ok[:n]
        else:
            out[perm[i0:i1]] = ok[:n]
    return out


if __name__ == "__main__":
    # small self-test with synthetic inputs (E scaled down)
    rng = np.random.default_rng(0)
    E, FX, FU, H, FO, B = 40960, 64, 64, 256, 64, 512
    src = rng.standard_normal((E, FX), dtype=np.float32)
    dest = rng.standard_normal((E, FX), dtype=np.float32)
    u = rng.standard_normal((B, FU), dtype=np.float32)
    batch = np.sort(rng.integers(0, B, E)).astype(np.int64)
    W1 = (rng.standard_normal((2 * FX + FU, H), dtype=np.float32) / np.sqrt(2 * FX + FU))
    b1 = np.zeros(H, np.float32)
    W2 = rng.standard_normal((H, FO), dtype=np.float32) / np.sqrt(H)
    b2 = rng.standard_normal(FO, dtype=np.float32)
    got = kernel(src=src, dest=dest, edge_attr=src, u=u, batch=batch,
                 W1=W1, b1=b1, W2=W2, b2=b2)
    x = np.concatenate([dest, src, u[batch]], axis=1)
    hh = np.maximum(x @ W1 + b1, 0.0)
    want = hh @ W2 + b2
    rel = np.linalg.norm(got - want) / np.linalg.norm(want)
    print("rel err:", rel)


# revision 26
# speedup vs baseline: 1.1932x; 1.0113x over previous
"""Trainium2 Bass kernel: EdgeModelConcat (GNN edge MLP).

reference math (per edge e):
    x   = concat([dest[e], src[e], u[batch[e]]])      # [192]
    h   = relu(x @ W1 + b1)                            # [256]
    out = h @ W2 + b2                                  # [64]
(edge_attr is an input but unused by the reference.)

Strategy
--------
Data-parallel over edges on 8 NeuronCores, all in bf16 with fp32 PSUM.
Host does all layout marshalling so the device only does DMAs + matmuls
+ fused bias/relu:

* host passes x^T = [dest^T; src^T] as a [128, E/8] bf16 array per core,
  so layer-1 is h^T = W1[:128].T @ x^T with K=128, no device transposes.
* the u-term is folded into a per-graph bias table computed on host:
  cT[g] = u[g] @ W1[128:] + b1 ([256, 512] f32, uploaded).  `batch` is
  sorted, so per 512-edge tile the bias column is piecewise constant;
  segment boundaries are baked into the instruction stream as static
  column ranges of the fused relu+bias ops.  Per-core segment structure
  differs -> one 8-way tc.Switch on partition_id.
* layer-2 accumulates the K=256 contraction as 2 matmuls per tile; even
  tiles write PSUM partitions 0:64 (PE col-group A), odd tiles 64:128
  (group B, via auto tile_position), so a pair of tiles lands as one
  [128, 512] PSUM tile evacuated in a single op and DMA'd at full
  partition width.  Layer-2 weight loads are shared across 2 pairs and
  layer-1 loads across 3 tiles (consecutive same-weight matmuls skip
  the ~107ns serialized LDWEIGHTS; measured, not just the docs' claim).
* dummy matmuls with no DMA dependency run at kernel start so the PE
  HAM clock-gate opens (1.2 -> 2.4 GHz) during the first slab load,
  which cannot land before ~13us (the DMA subsystem spends ~9.5us
  warming up and round-robins all queued transfers; ramp-phase slab
  loads are serialized via GpSimd gate ops so the first chunk is not
  starved by later slabs).
* outputs are stored bf16 and widened on the host.
"""

import numpy as np

MODE = "bf16"              # kept for test.py compat; only bf16 supported
PROFILE = False            # set True (with NTFF hook installed) to measure
LAST_EXEC_NS = None        # exec time of slowest profiled core, ns
LAST_RESULTS = None

NCORES = 8
TILE = 512                 # edges per matmul tile (PSUM bank = 512 f32)
SLAB_TILES = 8             # tiles per DMA slab (4096 edges = 1MB bf16 in)
WARMUP_MMS = 12            # dummy matmuls to open the HAM clock gate

_cache = {}


def _np_bf16():
    import ml_dtypes

    return np.dtype(ml_dtypes.bfloat16)


def _segments_per_tile(bk, ec, ntiles):
    """bk: per-core sorted graph ids [ec] -> list per tile of (a, b, g)."""
    out = []
    for t in range(ntiles):
        c0 = t * TILE
        w = min(TILE, ec - c0)
        vals = bk[c0 : c0 + w]
        bounds = np.flatnonzero(np.diff(vals)) + 1
        starts = np.concatenate([[0], bounds, [w]])
        out.append(
            [
                (int(starts[i]), int(starts[i + 1]), int(vals[starts[i]]))
                for i in range(len(starts) - 1)
            ]
        )
    return out


def _out_col(t):
    return (t // SLAB_TILES) * (SLAB_TILES // 2) * TILE + ((t % SLAB_TILES) // 2) * TILE


def _build(all_segs, ec, fx, fu, h, fo, b, out_w):
    from contextlib import ExitStack

    import concourse.bass as bass
    import concourse.mybir as mybir
    import concourse.tile as tile
    from concourse import bacc

    F32 = mybir.dt.float32
    BF16 = mybir.dt.bfloat16
    Relu = mybir.ActivationFunctionType.Relu
    Ident = mybir.ActivationFunctionType.Identity
    ADD = mybir.AluOpType.add
    MAX = mybir.AluOpType.max

    ntiles = (ec + TILE - 1) // TILE
    nslabs = (ntiles + SLAB_TILES - 1) // SLAB_TILES
    slab = TILE * SLAB_TILES
    kin = 2 * fx            # 128: contraction dim of layer 1
    mh = h // 128           # 2: H chunks of 128
    assert kin == 128 and h == 256 and fo <= 64

    nc = bacc.Bacc("TRN2", target_bir_lowering=False, debug=False, num_devices=NCORES)
    # cb (bf16) = [w1ds (h cols) | w2c (mh*fo cols)]
    # cbias (f32) = [cT chunk0 (b cols) | cT chunk1 (b cols) | b2 col]
    cb_w = h + mh * fo
    xT = nc.declare_dram_parameter("xT", [kin, ec], BF16, isOutput=False)
    cb = nc.declare_dram_parameter("cb", [128, cb_w], BF16, isOutput=False)
    cbias = nc.declare_dram_parameter("cbias", [128, mh * b + 1], F32, isOutput=False)
    outT = nc.declare_dram_parameter("outT", [128, out_w], BF16, isOutput=True)

    with tile.TileContext(nc) as tc, ExitStack() as ctx:
        pid = nc.partition_id()

        const = ctx.enter_context(tc.tile_pool(name="const", bufs=1))
        xp = ctx.enter_context(tc.tile_pool(name="xp", bufs=4))
        hp = ctx.enter_context(tc.tile_pool(name="hp", bufs=12))
        op = ctx.enter_context(tc.tile_pool(name="op", bufs=4))
        ph0 = ctx.enter_context(tc.tile_pool(name="ph0", bufs=3, space="PSUM"))
        ph1 = ctx.enter_context(tc.tile_pool(name="ph1", bufs=3, space="PSUM"))
        po = ctx.enter_context(tc.tile_pool(name="po", bufs=2, space="PSUM"))

        # PE warm-up: dummy matmuls with zero DMA dependencies keep the PE
        # busy from program start so the HAM clock-gate opens (~3.4us of
        # sustained activity) while the first input slab is still loading.
        wt = const.tile([128, TILE], BF16)
        nc.vector.memset(wt[:], 0.25)
        # dummy activation forces the ACT spline-table load (~1.3us) to
        # happen during the ramp instead of before the first real evac
        wact = const.tile([128, 64], BF16)
        nc.scalar.activation(wact[:], wt[:, 0:64], Relu, bias=0.0)
        for _ in range(WARMUP_MMS):
            wps = po.tile([128, TILE], F32, tag="o", name="wps")
            nc.tensor.matmul(wps[:], wt[:, 0:128], wt[:], start=True, stop=True)

        pre_slabs = {}   # slab index -> (in tile, out tile), common code

        def emit_slab_load(s, chunks=1, gate=None):
            # chunks>1 splits the load so the first tiles' data lands early.
            # gate=prev_xt serializes ramp-phase loads: DMA rings round-robin
            # across ALL queued transfers, so without a gate the first tile's
            # data crawls at 1/N of line rate behind later slabs.  The gate
            # is a 1-elem GpSimd copy (waits for the previous slab) followed
            # by a 1-elem memset into the new tile (WAW makes this slab's
            # DMA wait) -- GpSimd is otherwise idle, so nothing else blocks.
            c0 = s * slab
            ws = min(slab, ec - c0)
            xtn = xp.tile([kin, slab], BF16, tag="xt", name="xt")
            if gate is not None:
                gt = const.tile([1, 1], BF16)
                nc.gpsimd.tensor_copy(gt[:], gate[0:1, slab - 1 : slab])
                nc.gpsimd.memset(xtn[0:1, 0:1], 0)
            step = (slab // chunks + TILE - 1) // TILE * TILE
            for q0 in range(0, ws, step):
                q1 = min(q0 + step, ws)
                nc.sync.dma_start(
                    xtn[:, q0:q1], xT[:, c0 + q0 : c0 + q1]
                )
            otn = op.tile([128, slab // 2], BF16, tag="ot", name="ot")
            return (xtn, otn)

        # DMA issue order sets arrival order: weights (needed by the first
        # matmul), first input slab, bias table (needed ~1us later by the
        # first evac), then the prefetch slab.  Slabs 0/1 load in common
        # code; later slabs are issued inside each core's Switch branch.
        cb_sb = const.tile([128, cb_w], BF16)
        nc.sync.dma_start(cb_sb[:], cb[:])
        pre_slabs[0] = emit_slab_load(0, chunks=4)
        cbias_sb = const.tile([128, mh * b + 1], F32)
        nc.sync.dma_start(cbias_sb[:], cbias[:])
        if nslabs > 1:
            pre_slabs[1] = emit_slab_load(1, chunks=2, gate=pre_slabs[0][0])
        w1ds_sb = cb_sb[:, 0:h]
        w2c_sb = cb_sb[:, h : h + mh * fo]
        b2c_sb = cbias_sb[:, mh * b : mh * b + 1]

        for core in tc.Switch(pid, NCORES):
            segs_per_tile = all_segs[core]
            hss = {}       # tile index -> relu'd h tile (sbuf)
            widths = {}
            xts = {s: v[0] for s, v in pre_slabs.items()}
            ots = {s: v[1] for s, v in pre_slabs.items()}

            def load_slab(s):
                if s in xts or s >= nslabs:
                    return
                g = xts[s - 1] if s in (2, 3) else None
                xts[s], ots[s] = emit_slab_load(s, chunks=2, gate=g)

            def store_slab(tp, w, ot):
                # store in half-slabs (4 tiles = 256KB) so the final store
                # has less data queued behind it; the last slab stores per
                # pair so the tail drains as early as possible
                oc0 = (tp // SLAB_TILES) * (slab // 2)
                j = tp % SLAB_TILES
                if tp // SLAB_TILES == nslabs - 1:
                    cc = (j // 2) * TILE
                    if tp % 2 == 1:
                        nc.sync.dma_start(
                            outT[:, oc0 + cc : oc0 + cc + TILE],
                            ot[:, cc : cc + TILE],
                        )
                    elif tp == ntiles - 1:
                        nc.sync.dma_start(
                            outT[0:fo, oc0 + cc : oc0 + cc + w],
                            ot[0:fo, cc : cc + w],
                        )
                    return
                if j == 3:
                    nc.sync.dma_start(
                        outT[:, oc0 : oc0 + 2 * TILE], ot[:, 0 : 2 * TILE]
                    )
                elif j == SLAB_TILES - 1:
                    nc.sync.dma_start(
                        outT[:, oc0 + 2 * TILE : oc0 + 4 * TILE],
                        ot[:, 2 * TILE : 4 * TILE],
                    )
                elif tp == ntiles - 1:
                    lo = 2 * TILE if j >= 4 else 0
                    nt = j + 1
                    hi = (nt // 2) * TILE
                    if hi > lo:
                        nc.sync.dma_start(
                            outT[:, oc0 + lo : oc0 + hi], ot[:, lo:hi]
                        )
                    if nt % 2:
                        nc.sync.dma_start(
                            outT[0:fo, oc0 + hi : oc0 + hi + w],
                            ot[0:fo, hi : hi + w],
                        )

            # L1 for a group of 3 tiles: same-weight matmuls adjacent so the
            # walrus ldw-opt elides redundant LDWEIGHTS; relu+bias evac
            # split DVE (h0) / ACT (h1) per static batch segments.
            def emit_l1(group):
                tiles = []
                for t in group:
                    s, j = divmod(t, SLAB_TILES)
                    load_slab(s)
                    if j == 0:
                        load_slab(s + 1)   # prefetch a full slab ahead
                    if j == 4:
                        load_slab(s + 2)   # deepen prefetch mid-slab
                    xtt = xts[s]
                    a = j * TILE
                    w = min(TILE, ec - t * TILE)
                    widths[t] = w
                    h0 = ph0.tile([128, TILE], F32, tag="h0", name="h0")
                    h1 = ph1.tile([128, TILE], F32, tag="h1", name="h1")
                    tiles.append((t, xtt, a, w, h0, h1))
                for (t, xtt, a, w, h0, h1) in tiles:
                    nc.tensor.matmul(
                        h0[:, :w], w1ds_sb[:, 0:128], xtt[:, a : a + w],
                        start=True, stop=True,
                    )
                for (t, xtt, a, w, h0, h1) in tiles:
                    nc.tensor.matmul(
                        h1[:, :w], w1ds_sb[:, 128:256], xtt[:, a : a + w],
                        start=True, stop=True,
                    )
                for (t, xtt, a, w, h0, h1) in tiles:
                    hs = hp.tile([128, 2 * TILE], BF16, tag="hs", name="hs")
                    hss[t] = hs
                    for (sa, sb, g) in segs_per_tile[t]:
                        nc.vector.tensor_scalar(
                            out=hs[:, sa:sb], in0=h0[:, sa:sb],
                            scalar1=cbias_sb[:, g : g + 1], scalar2=0.0,
                            op0=ADD, op1=MAX,
                        )
                        nc.scalar.activation(
                            hs[:, TILE + sa : TILE + sb], h1[:, sa:sb], Relu,
                            bias=cbias_sb[:, b + g : b + g + 1],
                        )

            # L2 for a group of <=4 tiles (2 pairs): even tiles -> PSUM
            # partitions 0:64 (PE col-group A), odd -> 64:128 (group B).
            # Same-weight matmuls adjacent across the 2 pairs halve the
            # LDWEIGHTS count; accumulation pairs stay contiguous per
            # col-group (interleaving start/stop across groups serializes
            # the scheduler badly).
            def emit_l2(group):
                o_of = {}
                for p in range(0, len(group), 2):
                    opair = po.tile([128, TILE], F32, tag="o", name="o_pair")
                    for t in group[p : p + 2]:
                        o_of[t] = opair
                for r0, par in ((0, 0), (fo, 1)):       # col-group A, then B
                    gts = [t for t in group if t % 2 == par]
                    for m in range(mh):                 # W2 K-chunk 0, then 1
                        for t in gts:
                            w = widths[t]
                            nc.tensor.matmul(
                                o_of[t][r0 : r0 + fo, :w],
                                w2c_sb[:, m * fo : (m + 1) * fo],
                                hss[t][:, m * TILE : m * TILE + w],
                                start=(m == 0), stop=(m == mh - 1),
                            )
                for p in range(0, len(group), 2):
                    pt = group[p : p + 2]
                    t0 = pt[0]
                    full = len(pt) == 2
                    w = widths[pt[-1]]
                    for t in pt:
                        hss.pop(t), widths.pop(t)
                    s = t0 // SLAB_TILES
                    cc = ((t0 % SLAB_TILES) // 2) * TILE
                    ot = ots[s]
                    opair = o_of[t0]
                    if full:
                        src = opair[:]
                        dst = ot[:, cc : cc + TILE]
                    else:
                        src = opair[0:fo, :w]
                        dst = ot[0:fo, cc : cc + w]
                    if (t0 // 2) % 2 == 0:
                        nc.vector.tensor_scalar(
                            out=dst, in0=src,
                            scalar1=b2c_sb[:] if full else b2c_sb[0:fo, :],
                            scalar2=None, op0=ADD,
                        )
                    else:
                        nc.scalar.activation(
                            dst, src, Ident,
                            bias=b2c_sb[:] if full else b2c_sb[0:fo, :],
                        )
                    store_slab(pt[-1], w, ot)

            # L1 in groups of 3 (fits ph0/ph1 bufs, 2 LDW per 3 tiles);
            # L2 in groups of 4, lagging L1 by >=3 tiles for pipelining.
            l1_groups = [
                list(range(q, min(q + 3, ntiles))) for q in range(0, ntiles, 3)
            ]
            l2_groups = [
                list(range(q, min(q + 4, ntiles))) for q in range(0, ntiles, 4)
            ]
            next_l2 = 0
            for grp in l1_groups:
                emit_l1(grp)
                done = grp[-1] + 1
                while (
                    next_l2 < len(l2_groups)
                    and l2_groups[next_l2][-1] + 2 <= done
                ):
                    emit_l2(l2_groups[next_l2])
                    next_l2 += 1
            while next_l2 < len(l2_groups):
                emit_l2(l2_groups[next_l2])
                next_l2 += 1
    nc.compile()
    return nc


def kernel(**inputs):
    global LAST_EXEC_NS, LAST_RESULTS

    src = np.asarray(inputs["src"], dtype=np.float32)
    dest = np.asarray(inputs["dest"], dtype=np.float32)
    u = np.asarray(inputs["u"], dtype=np.float32)
    batch = np.asarray(inputs["batch"])
    W1 = np.asarray(inputs["W1"], dtype=np.float32)
    b1 = np.asarray(inputs["b1"], dtype=np.float32)
    W2 = np.asarray(inputs["W2"], dtype=np.float32)
    b2 = np.asarray(inputs["b2"], dtype=np.float32)

    e, fx = src.shape
    b_, fu = u.shape
    h = W1.shape[1]
    fo = W2.shape[1]
    ec = (e + NCORES - 1) // NCORES
    ntiles = (ec + TILE - 1) // TILE
    mh = h // 128
    bf16 = _np_bf16()

    # sorted edge order (identity when batch already sorted, as speced)
    bi = batch.astype(np.int64)
    if np.any(bi[1:] < bi[:-1]):
        perm = np.argsort(bi, kind="stable")
    else:
        perm = None
    bs = bi if perm is None else bi[perm]

    # host-side marshalling ------------------------------------------------
    # out column layout: tile t -> cols [S(t), S(t)+w) rows (t%2)*64
    wlast = ec - (ntiles - 1) * TILE
    out_w = max(_out_col(ntiles - 1) + wlast, _out_col(max(ntiles - 2, 0)) + TILE)

    # weights: cb = [W1[:128] | W2 chunks], bf16
    cb = np.concatenate(
        [W1[: 2 * fx]]
        + [np.concatenate([W2[i * 128 : (i + 1) * 128] for i in range(mh)], axis=1)],
        axis=1,
    ).astype(bf16)
    cb = np.ascontiguousarray(cb)

    # bias table: cT[g] = u[g] @ W1[128:] + b1, plus a b2 column, f32
    cT = u @ W1[2 * fx :] + b1                    # [b, h]
    cbias = np.zeros((128, mh * b_ + 1), dtype=np.float32)
    for m in range(mh):
        cbias[:, m * b_ : (m + 1) * b_] = cT[:, m * 128 : (m + 1) * 128].T
    cbias[:, mh * b_] = np.tile(b2, mh)[:128]

    all_segs = []
    in_maps = []
    for k in range(NCORES):
        i0, i1 = k * ec, min((k + 1) * ec, e)
        n = i1 - i0
        if perm is None:
            d_k = dest[i0:i1]
            s_k = src[i0:i1]
        else:
            idx = perm[i0:i1]
            d_k = dest[idx]
            s_k = src[idx]
        xTk = np.empty((2 * fx, ec), dtype=bf16)
        xTk[:fx, :n] = d_k.T
        xTk[fx:, :n] = s_k.T
        if n < ec:
            xTk[:, n:] = 0
        bk = np.empty(ec, dtype=np.int64)
        bk[:n] = bs[i0:i1]
        if n < ec:
            bk[n:] = bk[n - 1]
        all_segs.append(_segments_per_tile(bk, ec, ntiles))
        in_maps.append({"xT": xTk, "cb": cb, "cbias": cbias})

    # build / fetch compiled program --------------------------------------
    key = (e, fx, fu, h, fo, b_, hash(bs.tobytes()))
    nc = _cache.get(key)
    if nc is None:
        nc = _build(all_segs, ec, fx, fu, h, fo, b_, out_w)
        _cache.clear()
        _cache[key] = nc

    from concourse.bass_utils import run_bass_kernel_spmd

    res = run_bass_kernel_spmd(
        nc, in_maps, list(range(NCORES)), trace=bool(PROFILE)
    )
    LAST_EXEC_NS = res.exec_time_ns
    LAST_RESULTS = res

    # unpack ---------------------------------------------------------------
    out = np.empty((e, fo), dtype=np.float32)
    for k in range(NCORES):
        o = np.asarray(res.results[k]["outT"]).astype(np.float32)
        i0, i1 = k * ec, min((k + 1) * ec, e)
        n = i1 - i0
        ok = np.empty((ec, fo), dtype=np.float32)
        for t in range(ntiles):
            w = min(TILE, ec - t * TILE)
            c = _out_col(t)
            r = (t % 2) * 64
            ok[t * TILE : t * TILE + w] = o[r : r + fo, c : c + w].T
        if perm is None:
            out[i0:i1] = ok[:n]
        else:
            out[perm[i0:i1]] = ok[:n]
    return out


if __name__ == "__main__":
    # small self-test with synthetic inputs (E scaled down)
    rng = np.random.default_rng(0)
    E, FX, FU, H, FO, B = 40960, 64, 64, 256, 64, 512
    src = rng.standard_normal((E, FX), dtype=np.float32)
    dest = rng.standard_normal((E, FX), dtype=np.float32)
    u = rng.standard_normal((B, FU), dtype=np.float32)
    batch = np.sort(rng.integers(0, B, E)).astype(np.int64)
    W1 = (rng.standard_normal((2 * FX + FU, H), dtype=np.float32) / np.sqrt(2 * FX + FU))
    b1 = np.zeros(H, np.float32)
    W2 = rng.standard_normal((H, FO), dtype=np.float32) / np.sqrt(H)
    b2 = rng.standard_normal(FO, dtype=np.float32)
    got = kernel(src=src, dest=dest, edge_attr=src, u=u, batch=batch,
                 W1=W1, b1=b1, W2=W2, b2=b2)
    x = np.concatenate([dest, src, u[batch]], axis=1)
    hh = np.maximum(x @ W1 + b1, 0.0)
    want = hh @ W2 + b2
    rel = np.linalg.norm(got - want) / np.linalg.norm(want)
    print("rel err:", rel)
